# revision 11
# baseline (speedup 1.0000x reference)
"""Trainium2 Bass kernel for nn_MultiHeadedAttention_71425306132929.

Fused QKV projection + RoPE + causal/padding-masked SDPA + output projection.

Sharding: 8 cores = 2 batches x 4 head-groups (4 heads each).  Each core
computes, for its (batch, head-group):
    qkT = (Wq|Wk) @ query[b].T      (transposed layout: head-dim on partitions)
    RoPE on qT/kT via in-quadrant partition shuffle (head dims permuted
    host-side so RoPE partners are 16 partitions apart)
    scoresT[k,q] = kT.T-dot-qT per head (2 heads packed via PE row tiling)
    PT = exp(scoresT * 1/8)  (no max-subtraction needed: logits are O(1))
    causal masking: block-skip + affine_select on diagonal blocks
    padding mask: folded into v (zeroed rows) + an extra all-mask column that
    makes the attention-denominator fall out of the same matmul
    ohT = (v|m).T @ PT accumulated over key blocks -> unnormalized out + denom
    normalize via reciprocal_approx_fast + DMA partition-broadcast
    yT_partial = WoutT.T @ ohT  (row-parallel out-projection)
Host sums the 4 partial yT per batch.
"""

import os
import sys

import numpy as np

sys.path.insert(0, "/opt/trn_rl_repo")

import concourse.bass as bass  # noqa: E402
import concourse.bacc as bacc  # noqa: E402
import concourse.tile as tile  # noqa: E402
from concourse import mybir  # noqa: E402

import ml_dtypes  # noqa: E402

BF16 = mybir.dt.bfloat16
F32 = mybir.dt.float32

B, S, DM, TD, H, HD = 2, 2048, 1024, 1024, 16, 64
NCORES = 8
NH = 4          # heads per core
NKB = S // 128  # 16 key blocks
NQC = S // 512  # 4 query chunks
KC = DM // 128  # 8 contraction chunks

# RoPE partner permutation: place original dim d so that partner(p) = p ^ 16
# (within a 32-partition quadrant, reachable by DVE stream_shuffle).
ROPE_PERM = []
for _p in range(64):
    q32, r32 = _p // 32, _p % 32
    ROPE_PERM.append(q32 * 16 + r32 if r32 < 16 else 32 + q32 * 16 + (r32 - 16))
ROPE_SGN = np.array([-1.0 if (p % 32) < 16 else 1.0 for p in range(64)], np.float32)
SHUF_MASK = [i ^ 16 for i in range(32)]

_CACHED = {}


def build_program():
    nc = bacc.Bacc(None, target_bir_lowering=False)
    qT_d = nc.declare_dram_parameter("qT", [DM, S], BF16, isOutput=False)
    wqk_d = nc.declare_dram_parameter("wqkT", [DM, 512], BF16, isOutput=False)
    wv_d = nc.declare_dram_parameter("wvT", [DM, 256], BF16, isOutput=False)
    cos_d = nc.declare_dram_parameter("cosT", [128, S], BF16, isOutput=False)
    sin_d = nc.declare_dram_parameter("sinT", [128, S], BF16, isOutput=False)
    mkv_d = nc.declare_dram_parameter("maskv", [128, NKB], F32, isOutput=False)
    wo_d = nc.declare_dram_parameter("woutT", [256, DM], BF16, isOutput=False)
    yT_d = nc.declare_dram_parameter("yT", [DM, S], F32, isOutput=True)
    dscr = nc.dram_tensor("den_scratch", [16, 512], F32)

    with tile.TileContext(nc) as tc:
        with (
            tc.tile_pool(name="const", bufs=1) as cpool,
            tc.tile_pool(name="work", bufs=1) as wpool,
            tc.tile_pool(name="rope", bufs=3) as rpool,
            tc.tile_pool(name="pt", bufs=6) as ptpool,
            tc.tile_pool(name="nrm", bufs=3) as npool,
            tc.tile_pool(name="yout", bufs=2) as ypool,
            tc.tile_pool(name="psA", bufs=2, space="PSUM") as psA,
            tc.tile_pool(name="psP", bufs=2, space="PSUM") as psP,
            tc.tile_pool(name="psO", bufs=2, space="PSUM") as psO,
        ):
            qT_sb = cpool.tile([128, KC, S], BF16, tag="qT")
            wqk_sb = cpool.tile([128, KC, 512], BF16, tag="wqk")
            wv_sb = cpool.tile([128, KC, 256], BF16, tag="wv")
            cos_sb = cpool.tile([128, S], BF16, tag="cos")
            sin_sb = cpool.tile([128, S], BF16, tag="sin")
            mkv_sb = cpool.tile([128, NKB], F32, tag="mkv")
            wo_sb = cpool.tile([128, 2, DM], BF16, tag="wo")

            qk_sb = wpool.tile([128, 4, S], BF16, tag="qk")
            vaug_sb = wpool.tile([128, NKB, 4, 128], BF16, tag="vaug")
            ohT_sb = wpool.tile([128, 2, S], BF16, tag="ohT")

            nc.sync.dma_start(qT_sb[:], qT_d.rearrange("(c p) s -> p c s", p=128))
            nc.sync.dma_start(wqk_sb[:], wqk_d.rearrange("(c p) s -> p c s", p=128))
            nc.sync.dma_start(wv_sb[:], wv_d.rearrange("(c p) s -> p c s", p=128))
            nc.sync.dma_start(cos_sb[:], cos_d[:])
            nc.sync.dma_start(sin_sb[:], sin_d[:])
            nc.sync.dma_start(mkv_sb[:], mkv_d[:])
            nc.sync.dma_start(wo_sb[:], wo_d.rearrange("(c p) s -> p c s", p=128))

            nc.gpsimd.memset(vaug_sb[:], 0.0)
            # mask columns of v_aug: even slots col 64, odd slots col 32
            # (den must land on a legal engine start partition: 0/32/64/96)
            mkv_col = mkv_sb.rearrange("p (k o) -> p k o", o=1)
            nc.vector.tensor_copy(vaug_sb[:, :, 0, 64:65], mkv_col)
            nc.vector.tensor_copy(vaug_sb[:, :, 2, 64:65], mkv_col)
            nc.vector.tensor_copy(vaug_sb[:, :, 1, 32:33], mkv_col)
            nc.vector.tensor_copy(vaug_sb[:, :, 3, 32:33], mkv_col)

            def emit_qk(mt, qn):
                """project + rope one [128, 512] chunk of q or k (pair of heads)"""
                qsl = slice(qn * 512, qn * 512 + 512)
                ps = psP.tile([128, 512], F32, tag="psP")
                for kc in range(KC):
                    nc.tensor.matmul(
                        ps[:],
                        lhsT=wqk_sb[:, kc, mt * 128:(mt + 1) * 128],
                        rhs=qT_sb[:, kc, qsl],
                        start=(kc == 0),
                        stop=(kc == KC - 1),
                    )
                qkp = rpool.tile([128, 512], BF16, tag="qkp")
                nc.scalar.copy(qkp[:], ps[:])
                shuf = rpool.tile([128, 512], BF16, tag="shuf")
                nc.vector.stream_shuffle(shuf[:], qkp[:], mask=SHUF_MASK)
                t1 = rpool.tile([128, 512], BF16, tag="t1")
                nc.vector.tensor_mul(t1[:], qkp[:], cos_sb[:, qsl])
                t2 = rpool.tile([128, 512], BF16, tag="t2")
                nc.vector.tensor_mul(t2[:], shuf[:], sin_sb[:, qsl])
                nc.vector.tensor_add(qk_sb[:, mt, qsl], t1[:], t2[:])

            def emit_v(st):
                """project + mask one [128 keys, 4*64] v block into v_aug"""
                ps = psP.tile([128, 512], F32, tag="psP")
                psv = ps[:, 0:256]
                for kc in range(KC):
                    nc.tensor.matmul(
                        psv,
                        lhsT=qT_sb[:, kc, st * 128:(st + 1) * 128],
                        rhs=wv_sb[:, kc, :],
                        start=(kc == 0),
                        stop=(kc == KC - 1),
                    )
                psv_h = psv.rearrange("p (h d) -> p h d", h=4)
                msk = mkv_sb[:, st:st + 1]
                # even local heads (slots 0,2) -> cols 0:64 ; odd -> cols 64:128
                nc.vector.tensor_scalar_mul(vaug_sb[:, st, 0:4:2, 0:64], psv_h[:, 0:4:2, :], msk)
                nc.vector.tensor_scalar_mul(vaug_sb[:, st, 1:4:2, 64:128], psv_h[:, 1:4:2, :], msk)

            def emit_attn(pair, qc):
                nkb = 4 * qc + 4
                qmt, kmt = pair, 2 + pair
                qsl = slice(qc * 512, qc * 512 + 512)
                oT = [psO.tile([128, 512], F32, tag="psO", name=f"oT{_h}") for _h in range(2)]
                for kb in range(nkb):
                    ksl = slice(kb * 128, kb * 128 + 128)
                    st_ps = psA.tile([128, 1024], F32, tag="psA", name="stps")
                    for h in range(2):
                        pr = slice(64 * h, 64 * h + 64)
                        nc.tensor.matmul(
                            st_ps[:, h * 512:(h + 1) * 512],
                            lhsT=qk_sb[pr, kmt, ksl],
                            rhs=qk_sb[pr, qmt, qsl],
                            start=True,
                            stop=True,
                            skip_group_check=True,
                        )
                    pt = ptpool.tile([128, 1024], BF16, tag="pt", name="pt")
                    nc.scalar.activation(
                        pt[:], st_ps[:],
                        mybir.ActivationFunctionType.Exp,
                        scale=0.125,
                    )
                    joff = kb - 4 * qc
                    if joff >= 0:
                        co = joff * 128
                        for h in range(2):
                            if co > 0:
                                nc.gpsimd.memset(pt[:, h * 512:h * 512 + co], 0.0)
                            nc.gpsimd.affine_select(
                                pt[:, h * 512 + co:h * 512 + co + 128],
                                pt[:, h * 512 + co:h * 512 + co + 128],
                                pattern=[[1, 128]],
                                compare_op=mybir.AluOpType.is_ge,
                                fill=0.0,
                                base=0,
                                channel_multiplier=-1,
                            )
                    for h in range(2):
                        nc.tensor.matmul(
                            oT[h][:],
                            lhsT=vaug_sb[:, kb, 2 * pair + h, :],
                            rhs=pt[:, h * 512:(h + 1) * 512],
                            start=(kb == 0),
                            stop=(kb == nkb - 1),
                            skip_group_check=True,
                        )
                for h in range(2):
                    den_row = 64 if h == 0 else 32
                    r = slice(64 * h, 64 * h + 64)
                    den = npool.tile([128, 512], F32, tag="den")
                    nc.vector.reciprocal(den[den_row:den_row + 1, :], oT[h][den_row:den_row + 1, :])
                    # partition-broadcast via DRAM round-trip (gpsimd
                    # partition_broadcast mis-executes on HW; DMA from a DRAM
                    # source supports stride-0 partition APs)
                    idx = (pair * 4 + qc) * 2 + h
                    nc.sync.dma_start(dscr[idx:idx + 1, :], den[den_row:den_row + 1, :])
                    bc = npool.tile([128, 512], F32, tag="bc")
                    nc.sync.dma_start(bc[r, :], dscr[idx:idx + 1, :].to_broadcast((64, 512)))
                    nc.vector.tensor_mul(ohT_sb[r, pair, qsl], oT[h][r, :], bc[r, :])

            def emit_outproj(qn):
                qsl = slice(qn * 512, qn * 512 + 512)
                y = ypool.tile([128, 8, 512], F32, tag="y")
                for mt in range(8):
                    ps = psP.tile([128, 512], F32, tag="psP")
                    for kc2 in range(2):
                        nc.tensor.matmul(
                            ps[:],
                            lhsT=wo_sb[:, kc2, mt * 128:(mt + 1) * 128],
                            rhs=ohT_sb[:, kc2, qsl],
                            start=(kc2 == 0),
                            stop=(kc2 == 1),
                        )
                    nc.vector.tensor_copy(y[:, mt, :], ps[:])
                nc.sync.dma_start(yT_r[:, :, qsl], y[:])

            # pipeline by query chunk: project k/q/v for chunk qc, run both
            # head-pairs' attention, then the out-projection for that chunk
            # (keeps PE warm during the exp-paced attention phase).
            yT_r = yT_d.rearrange("(c p) s -> p c s", p=128)
            for qc in range(NQC):
                emit_qk(2, qc)
                emit_qk(3, qc)
                emit_qk(0, qc)
                emit_qk(1, qc)
                for st in range(4 * qc, 4 * qc + 4):
                    emit_v(st)
                emit_attn(0, qc)
                emit_attn(1, qc)
                emit_outproj(qc)

    nc.compile()
    return nc


def make_in_maps(query, W_in, W_out, sin_q, cos_q, attn_mask):
    bf = ml_dtypes.bfloat16
    cosT = np.asarray(cos_q, np.float32)[0, 0].T  # [64, S]
    sinT = np.asarray(sin_q, np.float32)[0, 0].T
    cosT_p = cosT[ROPE_PERM]
    sinT_p = sinT[ROPE_PERM] * ROPE_SGN[:, None]
    cos2 = np.concatenate([cosT_p, cosT_p], 0).astype(bf)    # [128, S]
    sin2 = np.concatenate([sinT_p, sinT_p], 0).astype(bf)
    W_in = np.asarray(W_in, np.float32)
    W_out = np.asarray(W_out, np.float32)
    query = np.asarray(query, np.float32)
    attn_mask = np.asarray(attn_mask)

    in_maps = []
    for c in range(NCORES):
        b, g = c // 4, c % 4
        heads = range(4 * g, 4 * g + 4)
        qrows = np.concatenate([W_in[h * 64:(h + 1) * 64][ROPE_PERM] for h in heads])
        krows = np.concatenate([W_in[TD + h * 64:TD + (h + 1) * 64][ROPE_PERM] for h in heads])
        vrows = np.concatenate([W_in[2 * TD + h * 64:2 * TD + (h + 1) * 64] for h in heads])
        tcols = np.concatenate([np.arange(h * 64, (h + 1) * 64) for h in heads])
        in_maps.append({
            "qT": np.ascontiguousarray(query[b].T).astype(bf),
            "wqkT": np.ascontiguousarray(np.concatenate([qrows, krows], 0).T).astype(bf),
            "wvT": np.ascontiguousarray(vrows.T).astype(bf),
            "cosT": cos2,
            "sinT": sin2,
            "maskv": np.ascontiguousarray(
                attn_mask[b].astype(np.float32).reshape(NKB, 128).T),
            "woutT": np.ascontiguousarray(W_out[:, tcols].T).astype(bf),
        })
    return in_maps


def _ensure_ntff_hook():
    """The image's antenv lacks axon_hooks; supply it so trace=True works."""
    try:
        from antenv.axon_hooks import get_axon_ntff_profile_hook  # noqa: F401
        return
    except ImportError:
        pass
    import types

    if "/root/.axon_site" not in sys.path:
        sys.path.insert(0, "/root/.axon_site")
    from trn_agent_boot.trn_boot import _ntff_profile_via_ctypes

    hook = _ntff_profile_via_ctypes("/opt/axon/libaxon_pjrt.so")
    mod = types.ModuleType("antenv.axon_hooks")
    mod._hook = hook
    mod.get_axon_ntff_profile_hook = lambda: mod._hook
    mod.set_axon_ntff_profile_hook = lambda h: setattr(mod, "_hook", h)
    sys.modules["antenv.axon_hooks"] = mod
    import antenv

    antenv.axon_hooks = mod


def kernel(query, W_in, W_out, sin_q, cos_q, attn_mask):
    if "nc" not in _CACHED:
        _CACHED["nc"] = build_program()
    nc = _CACHED["nc"]
    in_maps = make_in_maps(query, W_in, W_out, sin_q, cos_q, attn_mask)

    from concourse.bass_utils import run_bass_kernel_spmd

    trace = bool(os.environ.get("KERNEL_PROFILE"))
    if trace:
        try:
            _ensure_ntff_hook()
        except Exception as e:  # profiling is best-effort
            print(f"ntff hook unavailable: {e}")
            trace = False
    try:
        res = run_bass_kernel_spmd(nc, in_maps, list(range(NCORES)), trace=trace)
    except Exception:
        if not trace:
            raise
        res = run_bass_kernel_spmd(nc, in_maps, list(range(NCORES)), trace=False)
    _CACHED["last_result"] = res

    y = np.zeros((B, S, DM), np.float32)
    for c in range(NCORES):
        y[c // 4] += res.results[c]["yT"].T
    return y


# revision 12
# speedup vs baseline: 1.3474x; 1.3474x over previous
"""Trainium2 Bass kernel for nn_MultiHeadedAttention_71425306132929.

Fused QKV projection + RoPE + causal/padding-masked SDPA + output projection.

Sharding: 8 cores = 2 batches x 4 head-groups (4 heads each).  Each core
computes, for its (batch, head-group):
    qkT = (Wq|Wk) @ query[b].T      (transposed layout: head-dim on partitions)
    RoPE on qT/kT via in-quadrant partition shuffle (head dims permuted
    host-side so RoPE partners are 16 partitions apart)
    scoresT[k,q] = kT.T-dot-qT per head (2 heads packed via PE row tiling)
    PT = exp(scoresT * 1/8)  (no max-subtraction needed: logits are O(1))
    causal masking: block-skip + affine_select on diagonal blocks
    padding mask: folded into v (zeroed rows) + an extra all-mask column that
    makes the attention-denominator fall out of the same matmul
    ohT = (v|m).T @ PT accumulated over key blocks -> unnormalized out + denom
    normalize via reciprocal_approx_fast + DMA partition-broadcast
    yT_partial = WoutT.T @ ohT  (row-parallel out-projection)
Host sums the 4 partial yT per batch.
"""

import os
import sys

import numpy as np

sys.path.insert(0, "/opt/trn_rl_repo")

import concourse.bass as bass  # noqa: E402
import concourse.bacc as bacc  # noqa: E402
import concourse.tile as tile  # noqa: E402
from concourse import mybir  # noqa: E402

import ml_dtypes  # noqa: E402

BF16 = mybir.dt.bfloat16
F32 = mybir.dt.float32

B, S, DM, TD, H, HD = 2, 2048, 1024, 1024, 16, 64
NCORES = 8
NH = 4          # heads per core
NKB = S // 128  # 16 key blocks
NQC = S // 512  # 4 query chunks
KC = DM // 128  # 8 contraction chunks

# RoPE partner permutation: place original dim d so that partner(p) = p ^ 16
# (within a 32-partition quadrant, reachable by DVE stream_shuffle).
ROPE_PERM = []
for _p in range(64):
    q32, r32 = _p // 32, _p % 32
    ROPE_PERM.append(q32 * 16 + r32 if r32 < 16 else 32 + q32 * 16 + (r32 - 16))
ROPE_SGN = np.array([-1.0 if (p % 32) < 16 else 1.0 for p in range(64)], np.float32)
SHUF_MASK = [i ^ 16 for i in range(32)]

_CACHED = {}


def build_program():
    nc = bacc.Bacc(None, target_bir_lowering=False)
    qT_d = nc.declare_dram_parameter("qT", [DM, S], BF16, isOutput=False)
    wqk_d = nc.declare_dram_parameter("wqkT", [DM, 512], BF16, isOutput=False)
    wv_d = nc.declare_dram_parameter("wvT", [DM, 256], BF16, isOutput=False)
    cos_d = nc.declare_dram_parameter("cosT", [128, S], BF16, isOutput=False)
    sin_d = nc.declare_dram_parameter("sinT", [128, S], BF16, isOutput=False)
    mkv_d = nc.declare_dram_parameter("maskv", [128, NKB], F32, isOutput=False)
    wo_d = nc.declare_dram_parameter("woutT", [256, DM], BF16, isOutput=False)
    yT_d = nc.declare_dram_parameter("yT", [DM, S], F32, isOutput=True)
    dscr = nc.dram_tensor("den_scratch", [16, 512], F32)
    dscr2 = nc.dram_tensor("rcp_scratch", [16, 512], F32)

    with tile.TileContext(nc) as tc:
        with (
            tc.tile_pool(name="const", bufs=1) as cpool,
            tc.tile_pool(name="work", bufs=1) as wpool,
            tc.tile_pool(name="rope", bufs=3) as rpool,
            tc.tile_pool(name="pt", bufs=6) as ptpool,
            tc.tile_pool(name="nrm", bufs=4) as npool,
            tc.tile_pool(name="yout", bufs=2) as ypool,
            tc.tile_pool(name="psA", bufs=2, space="PSUM") as psA,
            tc.tile_pool(name="psP", bufs=2, space="PSUM") as psP,
            tc.tile_pool(name="psO", bufs=2, space="PSUM") as psO,
        ):
            qT_sb = cpool.tile([128, KC, S], BF16, tag="qT")
            wqk_sb = cpool.tile([128, KC, 512], BF16, tag="wqk")
            wv_sb = cpool.tile([128, KC, 256], BF16, tag="wv")
            cos_sb = cpool.tile([128, S], BF16, tag="cos")
            sin_sb = cpool.tile([128, S], BF16, tag="sin")
            mkv_sb = cpool.tile([128, NKB], F32, tag="mkv")
            wo_sb = cpool.tile([128, 2, DM], BF16, tag="wo")

            qk_sb = wpool.tile([128, 4, S], BF16, tag="qk")
            vaug_sb = wpool.tile([128, NKB, 4, 128], BF16, tag="vaug")
            ohT_sb = wpool.tile([128, 2, S], BF16, tag="ohT")

            nc.sync.dma_start(qT_sb[:], qT_d.rearrange("(c p) s -> p c s", p=128))
            nc.sync.dma_start(wqk_sb[:], wqk_d.rearrange("(c p) s -> p c s", p=128))
            nc.sync.dma_start(wv_sb[:], wv_d.rearrange("(c p) s -> p c s", p=128))
            nc.sync.dma_start(cos_sb[:], cos_d[:])
            nc.sync.dma_start(sin_sb[:], sin_d[:])
            nc.sync.dma_start(mkv_sb[:], mkv_d[:])
            nc.sync.dma_start(wo_sb[:], wo_d.rearrange("(c p) s -> p c s", p=128))

            nc.gpsimd.memset(vaug_sb[:], 0.0)
            # mask columns of v_aug: even slots col 64, odd slots col 32
            # (den must land on a legal engine start partition: 0/32/64/96)
            mkv_col = mkv_sb.rearrange("p (k o) -> p k o", o=1)
            nc.vector.tensor_copy(vaug_sb[:, :, 0, 64:65], mkv_col)
            nc.vector.tensor_copy(vaug_sb[:, :, 2, 64:65], mkv_col)
            nc.vector.tensor_copy(vaug_sb[:, :, 1, 32:33], mkv_col)
            nc.vector.tensor_copy(vaug_sb[:, :, 3, 32:33], mkv_col)

            def emit_qk(mt, qn):
                """project + rope one [128, 512] chunk of q or k (pair of heads)"""
                qsl = slice(qn * 512, qn * 512 + 512)
                ps = psP.tile([128, 512], F32, tag="psP")
                for kc in range(KC):
                    nc.tensor.matmul(
                        ps[:],
                        lhsT=wqk_sb[:, kc, mt * 128:(mt + 1) * 128],
                        rhs=qT_sb[:, kc, qsl],
                        start=(kc == 0),
                        stop=(kc == KC - 1),
                    )
                qkp = rpool.tile([128, 512], BF16, tag="qkp")
                nc.scalar.copy(qkp[:], ps[:])
                shuf = rpool.tile([128, 512], BF16, tag="shuf")
                nc.vector.stream_shuffle(shuf[:], qkp[:], mask=SHUF_MASK)
                t1 = rpool.tile([128, 512], BF16, tag="t1")
                nc.vector.tensor_mul(t1[:], qkp[:], cos_sb[:, qsl])
                t2 = rpool.tile([128, 512], BF16, tag="t2")
                nc.vector.tensor_mul(t2[:], shuf[:], sin_sb[:, qsl])
                nc.vector.tensor_add(qk_sb[:, mt, qsl], t1[:], t2[:])

            def emit_v(st):
                """project + mask one [128 keys, 4*64] v block into v_aug"""
                ps = psP.tile([128, 512], F32, tag="psP")
                psv = ps[:, 0:256]
                for kc in range(KC):
                    nc.tensor.matmul(
                        psv,
                        lhsT=qT_sb[:, kc, st * 128:(st + 1) * 128],
                        rhs=wv_sb[:, kc, :],
                        start=(kc == 0),
                        stop=(kc == KC - 1),
                    )
                psv_h = psv.rearrange("p (h d) -> p h d", h=4)
                msk = mkv_sb[:, st:st + 1]
                # even local heads (slots 0,2) -> cols 0:64 ; odd -> cols 64:128
                nc.vector.tensor_scalar_mul(vaug_sb[:, st, 0:4:2, 0:64], psv_h[:, 0:4:2, :], msk)
                nc.vector.tensor_scalar_mul(vaug_sb[:, st, 1:4:2, 64:128], psv_h[:, 1:4:2, :], msk)

            def emit_attn(pair, qc):
                nkb = 4 * qc + 4
                qmt, kmt = pair, 2 + pair
                qsl = slice(qc * 512, qc * 512 + 512)
                oT = [psO.tile([128, 512], F32, tag="psO", name=f"oT{_h}") for _h in range(2)]
                for kb in range(nkb):
                    ksl = slice(kb * 128, kb * 128 + 128)
                    st_ps = psA.tile([128, 1024], F32, tag="psA", name="stps")
                    for h in range(2):
                        pr = slice(64 * h, 64 * h + 64)
                        nc.tensor.matmul(
                            st_ps[:, h * 512:(h + 1) * 512],
                            lhsT=qk_sb[pr, kmt, ksl],
                            rhs=qk_sb[pr, qmt, qsl],
                            start=True,
                            stop=True,
                            skip_group_check=True,
                        )
                    pt = ptpool.tile([128, 1024], BF16, tag="pt", name="pt")
                    nc.scalar.activation(
                        pt[:], st_ps[:],
                        mybir.ActivationFunctionType.Exp,
                        scale=0.125,
                    )
                    joff = kb - 4 * qc
                    if joff >= 0:
                        co = joff * 128
                        for h in range(2):
                            if co > 0:
                                nc.gpsimd.memset(pt[:, h * 512:h * 512 + co], 0.0)
                            nc.gpsimd.affine_select(
                                pt[:, h * 512 + co:h * 512 + co + 128],
                                pt[:, h * 512 + co:h * 512 + co + 128],
                                pattern=[[1, 128]],
                                compare_op=mybir.AluOpType.is_ge,
                                fill=0.0,
                                base=0,
                                channel_multiplier=-1,
                            )
                    for h in range(2):
                        nc.tensor.matmul(
                            oT[h][:],
                            lhsT=vaug_sb[:, kb, 2 * pair + h, :],
                            rhs=pt[:, h * 512:(h + 1) * 512],
                            start=(kb == 0),
                            stop=(kb == nkb - 1),
                            skip_group_check=True,
                        )
                base = (pair * 4 + qc) * 2
                osb = []
                for h in range(2):
                    den_row = 64 if h == 0 else 32
                    o = npool.tile([128, 512], F32, tag="osb", name=f"osb{h}")
                    nc.vector.tensor_copy(o[:], oT[h][:])  # frees the psum bank
                    osb.append(o)
                    nc.sync.dma_start(dscr[base + h:base + h + 1, :], o[den_row:den_row + 1, :])
                # reciprocal on a partition-packed view (DMA reshape through
                # DRAM): [2, 512] dens -> [128, 8] -> 1/x -> back, then
                # partition-broadcast each head's 512 recips to 64 rows.
                rcp = npool.tile([128, 8], F32, tag="rcp")
                nc.sync.dma_start(rcp[:], dscr[base:base + 2, :].rearrange("a (p f) -> (a p) f", f=8))
                rcp2 = npool.tile([128, 8], F32, tag="rcp2")
                nc.vector.reciprocal(rcp2[:], rcp[:])
                nc.sync.dma_start(dscr2[base:base + 2, :].rearrange("a (p f) -> (a p) f", f=8), rcp2[:])
                bc = npool.tile([128, 512], F32, tag="bc")
                nc.sync.dma_start(bc[0:64, :], dscr2[base:base + 1, :].to_broadcast((64, 512)))
                nc.sync.dma_start(bc[64:128, :], dscr2[base + 1:base + 2, :].to_broadcast((64, 512)))
                nc.vector.tensor_mul(ohT_sb[0:64, pair, qsl], osb[0][0:64, :], bc[0:64, :])
                nc.vector.tensor_mul(ohT_sb[64:128, pair, qsl], osb[1][64:128, :], bc[64:128, :])

            def emit_outproj(qn):
                qsl = slice(qn * 512, qn * 512 + 512)
                y = ypool.tile([128, 8, 512], F32, tag="y")
                for mt in range(8):
                    ps = psP.tile([128, 512], F32, tag="psP")
                    for kc2 in range(2):
                        nc.tensor.matmul(
                            ps[:],
                            lhsT=wo_sb[:, kc2, mt * 128:(mt + 1) * 128],
                            rhs=ohT_sb[:, kc2, qsl],
                            start=(kc2 == 0),
                            stop=(kc2 == 1),
                        )
                    nc.vector.tensor_copy(y[:, mt, :], ps[:])
                nc.sync.dma_start(yT_r[:, :, qsl], y[:])

            # pipeline by query chunk: project k/q/v for chunk qc, run both
            # head-pairs' attention, then the out-projection for that chunk
            # (keeps PE warm during the exp-paced attention phase).
            yT_r = yT_d.rearrange("(c p) s -> p c s", p=128)
            for qc in range(NQC):
                emit_qk(2, qc)
                emit_qk(3, qc)
                emit_qk(0, qc)
                emit_qk(1, qc)
                for st in range(4 * qc, 4 * qc + 4):
                    emit_v(st)
                emit_attn(0, qc)
                emit_attn(1, qc)
                emit_outproj(qc)

    nc.compile()
    return nc


def make_in_maps(query, W_in, W_out, sin_q, cos_q, attn_mask):
    bf = ml_dtypes.bfloat16
    cosT = np.asarray(cos_q, np.float32)[0, 0].T  # [64, S]
    sinT = np.asarray(sin_q, np.float32)[0, 0].T
    cosT_p = cosT[ROPE_PERM]
    sinT_p = sinT[ROPE_PERM] * ROPE_SGN[:, None]
    cos2 = np.concatenate([cosT_p, cosT_p], 0).astype(bf)    # [128, S]
    sin2 = np.concatenate([sinT_p, sinT_p], 0).astype(bf)
    W_in = np.asarray(W_in, np.float32)
    W_out = np.asarray(W_out, np.float32)
    query = np.asarray(query, np.float32)
    attn_mask = np.asarray(attn_mask)

    in_maps = []
    for c in range(NCORES):
        b, g = c // 4, c % 4
        heads = range(4 * g, 4 * g + 4)
        qrows = np.concatenate([W_in[h * 64:(h + 1) * 64][ROPE_PERM] for h in heads])
        krows = np.concatenate([W_in[TD + h * 64:TD + (h + 1) * 64][ROPE_PERM] for h in heads])
        vrows = np.concatenate([W_in[2 * TD + h * 64:2 * TD + (h + 1) * 64] for h in heads])
        tcols = np.concatenate([np.arange(h * 64, (h + 1) * 64) for h in heads])
        in_maps.append({
            "qT": np.ascontiguousarray(query[b].T).astype(bf),
            "wqkT": np.ascontiguousarray(np.concatenate([qrows, krows], 0).T).astype(bf),
            "wvT": np.ascontiguousarray(vrows.T).astype(bf),
            "cosT": cos2,
            "sinT": sin2,
            "maskv": np.ascontiguousarray(
                attn_mask[b].astype(np.float32).reshape(NKB, 128).T),
            "woutT": np.ascontiguousarray(W_out[:, tcols].T).astype(bf),
        })
    return in_maps


def _ensure_ntff_hook():
    """The image's antenv lacks axon_hooks; supply it so trace=True works."""
    try:
        from antenv.axon_hooks import get_axon_ntff_profile_hook  # noqa: F401
        return
    except ImportError:
        pass
    import types

    if "/root/.axon_site" not in sys.path:
        sys.path.insert(0, "/root/.axon_site")
    from trn_agent_boot.trn_boot import _ntff_profile_via_ctypes

    hook = _ntff_profile_via_ctypes("/opt/axon/libaxon_pjrt.so")
    mod = types.ModuleType("antenv.axon_hooks")
    mod._hook = hook
    mod.get_axon_ntff_profile_hook = lambda: mod._hook
    mod.set_axon_ntff_profile_hook = lambda h: setattr(mod, "_hook", h)
    sys.modules["antenv.axon_hooks"] = mod
    import antenv

    antenv.axon_hooks = mod


def kernel(query, W_in, W_out, sin_q, cos_q, attn_mask):
    if "nc" not in _CACHED:
        _CACHED["nc"] = build_program()
    nc = _CACHED["nc"]
    in_maps = make_in_maps(query, W_in, W_out, sin_q, cos_q, attn_mask)

    from concourse.bass_utils import run_bass_kernel_spmd

    trace = bool(os.environ.get("KERNEL_PROFILE"))
    if trace:
        try:
            _ensure_ntff_hook()
        except Exception as e:  # profiling is best-effort
            print(f"ntff hook unavailable: {e}")
            trace = False
    try:
        res = run_bass_kernel_spmd(nc, in_maps, list(range(NCORES)), trace=trace)
    except Exception:
        if not trace:
            raise
        res = run_bass_kernel_spmd(nc, in_maps, list(range(NCORES)), trace=False)
    _CACHED["last_result"] = res

    y = np.zeros((B, S, DM), np.float32)
    for c in range(NCORES):
        y[c // 4] += res.results[c]["yT"].T
    return y


# revision 13
# speedup vs baseline: 1.5965x; 1.1849x over previous
"""Trainium2 Bass kernel for nn_MultiHeadedAttention_71425306132929.

Fused QKV projection + RoPE + causal/padding-masked SDPA + output projection.

Sharding: 8 cores = 2 batches x 4 head-groups (4 heads each).  Each core
computes, for its (batch, head-group):
    qkT = (Wq|Wk) @ query[b].T      (transposed layout: head-dim on partitions)
    RoPE on qT/kT via in-quadrant partition shuffle (head dims permuted
    host-side so RoPE partners are 16 partitions apart)
    scoresT[k,q] = kT.T-dot-qT per head (2 heads packed via PE row tiling)
    PT = exp(scoresT * 1/8)  (no max-subtraction needed: logits are O(1))
    causal masking: block-skip + affine_select on diagonal blocks
    padding mask: folded into v (zeroed rows) + an extra all-mask column that
    makes the attention-denominator fall out of the same matmul
    ohT = (v|m).T @ PT accumulated over key blocks -> unnormalized out + denom
    normalize via reciprocal_approx_fast + DMA partition-broadcast
    yT_partial = WoutT.T @ ohT  (row-parallel out-projection)
Host sums the 4 partial yT per batch.
"""

import os
import sys

import numpy as np

sys.path.insert(0, "/opt/trn_rl_repo")

import concourse.bass as bass  # noqa: E402
import concourse.bacc as bacc  # noqa: E402
import concourse.tile as tile  # noqa: E402
from concourse import mybir  # noqa: E402

import ml_dtypes  # noqa: E402

BF16 = mybir.dt.bfloat16
F32 = mybir.dt.float32

B, S, DM, TD, H, HD = 2, 2048, 1024, 1024, 16, 64
NCORES = 8
NH = 4          # heads per core
NKB = S // 128  # 16 key blocks
NQC = S // 512  # 4 query chunks
KC = DM // 128  # 8 contraction chunks

# RoPE partner permutation: place original dim d so that partner(p) = p ^ 16
# (within a 32-partition quadrant, reachable by DVE stream_shuffle).
ROPE_PERM = []
for _p in range(64):
    q32, r32 = _p // 32, _p % 32
    ROPE_PERM.append(q32 * 16 + r32 if r32 < 16 else 32 + q32 * 16 + (r32 - 16))
ROPE_SGN = np.array([-1.0 if (p % 32) < 16 else 1.0 for p in range(64)], np.float32)
SHUF_MASK = [i ^ 16 for i in range(32)]

_CACHED = {}


def build_program():
    nc = bacc.Bacc(None, target_bir_lowering=False)
    qT_d = nc.declare_dram_parameter("qT", [DM, S], BF16, isOutput=False)
    wqk_d = nc.declare_dram_parameter("wqkT", [DM, 512], BF16, isOutput=False)
    wv_d = nc.declare_dram_parameter("wvT", [DM, 256], BF16, isOutput=False)
    cos_d = nc.declare_dram_parameter("cosT", [128, S], BF16, isOutput=False)
    sin_d = nc.declare_dram_parameter("sinT", [128, S], BF16, isOutput=False)
    mkv_d = nc.declare_dram_parameter("maskv", [128, NKB], F32, isOutput=False)
    wo_d = nc.declare_dram_parameter("woutT", [256, DM], BF16, isOutput=False)
    yT_d = nc.declare_dram_parameter("yT", [DM, S], F32, isOutput=True)
    dscr = nc.dram_tensor("den_scratch", [16, 512], F32)
    dscr2 = nc.dram_tensor("rcp_scratch", [16, 512], F32)

    with tile.TileContext(nc) as tc:
        with (
            tc.tile_pool(name="const", bufs=1) as cpool,
            tc.tile_pool(name="work", bufs=1) as wpool,
            tc.tile_pool(name="rope", bufs=3) as rpool,
            tc.tile_pool(name="pt", bufs=6) as ptpool,
            tc.tile_pool(name="nrm", bufs=4) as npool,
            tc.tile_pool(name="yout", bufs=2) as ypool,
            tc.tile_pool(name="psA", bufs=2, space="PSUM") as psA,
            tc.tile_pool(name="psP", bufs=2, space="PSUM") as psP,
            tc.tile_pool(name="psO", bufs=2, space="PSUM") as psO,
        ):
            qT_sb = cpool.tile([128, KC, S], BF16, tag="qT")
            wqk_sb = cpool.tile([128, KC, 512], BF16, tag="wqk")
            wv_sb = cpool.tile([128, KC, 256], BF16, tag="wv")
            cos_sb = cpool.tile([128, S], BF16, tag="cos")
            sin_sb = cpool.tile([128, S], BF16, tag="sin")
            mkv_sb = cpool.tile([128, NKB], F32, tag="mkv")
            wo_sb = cpool.tile([128, 2, DM], BF16, tag="wo")

            qk_sb = wpool.tile([128, 4, S], BF16, tag="qk")
            vaug_sb = wpool.tile([128, NKB, 4, 128], BF16, tag="vaug")
            ohT_sb = wpool.tile([128, 2, S], BF16, tag="ohT")

            nc.sync.dma_start(wqk_sb[:], wqk_d.rearrange("(c p) s -> p c s", p=128))
            qT_r = qT_d.rearrange("(c p) s -> p c s", p=128)
            for kc in range(KC):
                nc.sync.dma_start(qT_sb[:, kc, :], qT_r[:, kc, :])
            nc.sync.dma_start(wv_sb[:], wv_d.rearrange("(c p) s -> p c s", p=128))
            nc.sync.dma_start(cos_sb[:], cos_d[:])
            nc.sync.dma_start(sin_sb[:], sin_d[:])
            nc.sync.dma_start(mkv_sb[:], mkv_d[:])
            nc.sync.dma_start(wo_sb[:], wo_d.rearrange("(c p) s -> p c s", p=128))

            nc.gpsimd.memset(vaug_sb[:], 0.0)
            # mask columns of v_aug: even slots col 64, odd slots col 32
            # (den must land on a legal engine start partition: 0/32/64/96)
            mkv_col = mkv_sb.rearrange("p (k o) -> p k o", o=1)
            nc.vector.tensor_copy(vaug_sb[:, :, 0, 64:65], mkv_col)
            nc.vector.tensor_copy(vaug_sb[:, :, 2, 64:65], mkv_col)
            nc.vector.tensor_copy(vaug_sb[:, :, 1, 32:33], mkv_col)
            nc.vector.tensor_copy(vaug_sb[:, :, 3, 32:33], mkv_col)

            def emit_qk(mt, qn):
                """project + rope one [128, 512] chunk of q or k (pair of heads)"""
                qsl = slice(qn * 512, qn * 512 + 512)
                ps = psP.tile([128, 512], F32, tag="psP")
                for kc in range(KC):
                    nc.tensor.matmul(
                        ps[:],
                        lhsT=wqk_sb[:, kc, mt * 128:(mt + 1) * 128],
                        rhs=qT_sb[:, kc, qsl],
                        start=(kc == 0),
                        stop=(kc == KC - 1),
                    )
                qkp = rpool.tile([128, 512], BF16, tag="qkp")
                nc.vector.tensor_copy(qkp[:], ps[:])
                shuf = rpool.tile([128, 512], BF16, tag="shuf")
                nc.vector.stream_shuffle(shuf[:], qkp[:], mask=SHUF_MASK)
                t1 = rpool.tile([128, 512], BF16, tag="t1")
                nc.vector.tensor_mul(t1[:], qkp[:], cos_sb[:, qsl])
                t2 = rpool.tile([128, 512], BF16, tag="t2")
                nc.vector.tensor_mul(t2[:], shuf[:], sin_sb[:, qsl])
                nc.vector.tensor_add(qk_sb[:, mt, qsl], t1[:], t2[:])

            def emit_v(st):
                """project + mask one [128 keys, 4*64] v block into v_aug"""
                ps = psP.tile([128, 512], F32, tag="psP")
                psv = ps[:, 0:256]
                for kc in range(KC):
                    nc.tensor.matmul(
                        psv,
                        lhsT=qT_sb[:, kc, st * 128:(st + 1) * 128],
                        rhs=wv_sb[:, kc, :],
                        start=(kc == 0),
                        stop=(kc == KC - 1),
                    )
                psv_h = psv.rearrange("p (h d) -> p h d", h=4)
                msk = mkv_sb[:, st:st + 1]
                # even local heads (slots 0,2) -> cols 0:64 ; odd -> cols 64:128
                nc.vector.tensor_scalar_mul(vaug_sb[:, st, 0:4:2, 0:64], psv_h[:, 0:4:2, :], msk)
                nc.vector.tensor_scalar_mul(vaug_sb[:, st, 1:4:2, 64:128], psv_h[:, 1:4:2, :], msk)

            def emit_attn(pair, qc):
                nkb = 4 * qc + 4
                qmt, kmt = pair, 2 + pair
                qsl = slice(qc * 512, qc * 512 + 512)
                oT = [psO.tile([128, 512], F32, tag="psO", name=f"oT{_h}") for _h in range(2)]
                for kb in range(nkb):
                    ksl = slice(kb * 128, kb * 128 + 128)
                    st_ps = psA.tile([128, 1024], F32, tag="psA", name="stps")
                    for h in range(2):
                        pr = slice(64 * h, 64 * h + 64)
                        nc.tensor.matmul(
                            st_ps[:, h * 512:(h + 1) * 512],
                            lhsT=qk_sb[pr, kmt, ksl],
                            rhs=qk_sb[pr, qmt, qsl],
                            start=True,
                            stop=True,
                            skip_group_check=True,
                        )
                    pt = ptpool.tile([128, 1024], BF16, tag="pt", name="pt")
                    nc.scalar.activation(
                        pt[:], st_ps[:],
                        mybir.ActivationFunctionType.Exp,
                        scale=0.125,
                    )
                    joff = kb - 4 * qc
                    if joff >= 0:
                        co = joff * 128
                        for h in range(2):
                            if co > 0:
                                nc.gpsimd.memset(pt[:, h * 512:h * 512 + co], 0.0)
                            nc.gpsimd.affine_select(
                                pt[:, h * 512 + co:h * 512 + co + 128],
                                pt[:, h * 512 + co:h * 512 + co + 128],
                                pattern=[[1, 128]],
                                compare_op=mybir.AluOpType.is_ge,
                                fill=0.0,
                                base=0,
                                channel_multiplier=-1,
                            )
                    for h in range(2):
                        nc.tensor.matmul(
                            oT[h][:],
                            lhsT=vaug_sb[:, kb, 2 * pair + h, :],
                            rhs=pt[:, h * 512:(h + 1) * 512],
                            start=(kb == 0),
                            stop=(kb == nkb - 1),
                            skip_group_check=True,
                        )
                base = (pair * 4 + qc) * 2
                osb = []
                for h in range(2):
                    den_row = 64 if h == 0 else 32
                    o = npool.tile([128, 512], F32, tag="osb", name=f"osb{h}")
                    nc.vector.tensor_copy(o[:], oT[h][:])  # frees the psum bank
                    osb.append(o)
                    nc.sync.dma_start(dscr[base + h:base + h + 1, :], o[den_row:den_row + 1, :])
                # reciprocal on a partition-packed view (DMA reshape through
                # DRAM): [2, 512] dens -> [128, 8] -> 1/x -> back, then
                # partition-broadcast each head's 512 recips to 64 rows.
                rcp = npool.tile([128, 8], F32, tag="rcp")
                nc.sync.dma_start(rcp[:], dscr[base:base + 2, :].rearrange("a (p f) -> (a p) f", f=8))
                rcp2 = npool.tile([128, 8], F32, tag="rcp2")
                nc.vector.reciprocal(rcp2[:], rcp[:])
                nc.sync.dma_start(dscr2[base:base + 2, :].rearrange("a (p f) -> (a p) f", f=8), rcp2[:])
                bc = npool.tile([128, 512], F32, tag="bc")
                nc.sync.dma_start(bc[0:64, :], dscr2[base:base + 1, :].to_broadcast((64, 512)))
                nc.sync.dma_start(bc[64:128, :], dscr2[base + 1:base + 2, :].to_broadcast((64, 512)))
                nc.vector.tensor_mul(ohT_sb[0:64, pair, qsl], osb[0][0:64, :], bc[0:64, :])
                nc.vector.tensor_mul(ohT_sb[64:128, pair, qsl], osb[1][64:128, :], bc[64:128, :])

            def emit_outproj(qn):
                qsl = slice(qn * 512, qn * 512 + 512)
                y = ypool.tile([128, 8, 512], F32, tag="y")
                for mt in range(8):
                    ps = psP.tile([128, 512], F32, tag="psP")
                    for kc2 in range(2):
                        nc.tensor.matmul(
                            ps[:],
                            lhsT=wo_sb[:, kc2, mt * 128:(mt + 1) * 128],
                            rhs=ohT_sb[:, kc2, qsl],
                            start=(kc2 == 0),
                            stop=(kc2 == 1),
                        )
                    nc.vector.tensor_copy(y[:, mt, :], ps[:])
                nc.sync.dma_start(yT_r[:, :, qsl], y[:])

            # pipeline by query chunk.  Emission order = scheduling
            # priority: next-chunk projections go between the two attention
            # blocks so they execute during the exp-paced stream (PE filler)
            # and are ready the moment the previous chunk drains; the
            # out-projection fills the second half of each round.
            yT_r = yT_d.rearrange("(c p) s -> p c s", p=128)

            def emit_prep(qc):
                emit_qk(2, qc)
                emit_qk(3, qc)
                emit_qk(0, qc)
                emit_qk(1, qc)
                for st in range(4 * qc, 4 * qc + 4):
                    emit_v(st)

            emit_prep(0)
            for qc in range(NQC):
                emit_attn(0, qc)
                if qc + 1 < NQC:
                    emit_prep(qc + 1)
                emit_attn(1, qc)
                emit_outproj(qc)

    nc.compile()
    return nc


def make_in_maps(query, W_in, W_out, sin_q, cos_q, attn_mask):
    bf = ml_dtypes.bfloat16
    cosT = np.asarray(cos_q, np.float32)[0, 0].T  # [64, S]
    sinT = np.asarray(sin_q, np.float32)[0, 0].T
    cosT_p = cosT[ROPE_PERM]
    sinT_p = sinT[ROPE_PERM] * ROPE_SGN[:, None]
    cos2 = np.concatenate([cosT_p, cosT_p], 0).astype(bf)    # [128, S]
    sin2 = np.concatenate([sinT_p, sinT_p], 0).astype(bf)
    W_in = np.asarray(W_in, np.float32)
    W_out = np.asarray(W_out, np.float32)
    query = np.asarray(query, np.float32)
    attn_mask = np.asarray(attn_mask)

    in_maps = []
    for c in range(NCORES):
        b, g = c // 4, c % 4
        heads = range(4 * g, 4 * g + 4)
        qrows = np.concatenate([W_in[h * 64:(h + 1) * 64][ROPE_PERM] for h in heads])
        krows = np.concatenate([W_in[TD + h * 64:TD + (h + 1) * 64][ROPE_PERM] for h in heads])
        vrows = np.concatenate([W_in[2 * TD + h * 64:2 * TD + (h + 1) * 64] for h in heads])
        tcols = np.concatenate([np.arange(h * 64, (h + 1) * 64) for h in heads])
        in_maps.append({
            "qT": np.ascontiguousarray(query[b].T).astype(bf),
            "wqkT": np.ascontiguousarray(np.concatenate([qrows, krows], 0).T).astype(bf),
            "wvT": np.ascontiguousarray(vrows.T).astype(bf),
            "cosT": cos2,
            "sinT": sin2,
            "maskv": np.ascontiguousarray(
                attn_mask[b].astype(np.float32).reshape(NKB, 128).T),
            "woutT": np.ascontiguousarray(W_out[:, tcols].T).astype(bf),
        })
    return in_maps


def _ensure_ntff_hook():
    """The image's antenv lacks axon_hooks; supply it so trace=True works."""
    try:
        from antenv.axon_hooks import get_axon_ntff_profile_hook  # noqa: F401
        return
    except ImportError:
        pass
    import types

    if "/root/.axon_site" not in sys.path:
        sys.path.insert(0, "/root/.axon_site")
    from trn_agent_boot.trn_boot import _ntff_profile_via_ctypes

    hook = _ntff_profile_via_ctypes("/opt/axon/libaxon_pjrt.so")
    mod = types.ModuleType("antenv.axon_hooks")
    mod._hook = hook
    mod.get_axon_ntff_profile_hook = lambda: mod._hook
    mod.set_axon_ntff_profile_hook = lambda h: setattr(mod, "_hook", h)
    sys.modules["antenv.axon_hooks"] = mod
    import antenv

    antenv.axon_hooks = mod


def kernel(query, W_in, W_out, sin_q, cos_q, attn_mask):
    if "nc" not in _CACHED:
        _CACHED["nc"] = build_program()
    nc = _CACHED["nc"]
    in_maps = make_in_maps(query, W_in, W_out, sin_q, cos_q, attn_mask)

    from concourse.bass_utils import run_bass_kernel_spmd

    trace = bool(os.environ.get("KERNEL_PROFILE"))
    if trace:
        try:
            _ensure_ntff_hook()
        except Exception as e:  # profiling is best-effort
            print(f"ntff hook unavailable: {e}")
            trace = False
    try:
        res = run_bass_kernel_spmd(nc, in_maps, list(range(NCORES)), trace=trace)
    except Exception:
        if not trace:
            raise
        res = run_bass_kernel_spmd(nc, in_maps, list(range(NCORES)), trace=False)
    _CACHED["last_result"] = res

    y = np.zeros((B, S, DM), np.float32)
    for c in range(NCORES):
        y[c // 4] += res.results[c]["yT"].T
    return y


# revision 15
# speedup vs baseline: 1.6871x; 1.0567x over previous
"""Trainium2 Bass kernel for nn_MultiHeadedAttention_71425306132929.

Fused QKV projection + RoPE + causal/padding-masked SDPA + output projection.

Sharding: 8 cores = 2 batches x 4 head-groups (4 heads each).  Each core
computes, for its (batch, head-group):
    qkT = (Wq|Wk) @ query[b].T      (transposed layout: head-dim on partitions)
    RoPE on qT/kT via in-quadrant partition shuffle (head dims permuted
    host-side so RoPE partners are 16 partitions apart)
    scoresT[k,q] = kT.T-dot-qT per head (2 heads packed via PE row tiling)
    PT = exp(scoresT * 1/8)  (no max-subtraction needed: logits are O(1))
    causal masking: block-skip + affine_select on diagonal blocks
    padding mask: folded into v (zeroed rows) + an extra all-mask column that
    makes the attention-denominator fall out of the same matmul
    ohT = (v|m).T @ PT accumulated over key blocks -> unnormalized out + denom
    normalize via reciprocal_approx_fast + DMA partition-broadcast
    yT_partial = WoutT.T @ ohT  (row-parallel out-projection)
Host sums the 4 partial yT per batch.
"""

import os
import sys

import numpy as np

sys.path.insert(0, "/opt/trn_rl_repo")

import concourse.bass as bass  # noqa: E402
import concourse.bacc as bacc  # noqa: E402
import concourse.tile as tile  # noqa: E402
from concourse import mybir  # noqa: E402

import ml_dtypes  # noqa: E402

BF16 = mybir.dt.bfloat16
F32 = mybir.dt.float32

B, S, DM, TD, H, HD = 2, 2048, 1024, 1024, 16, 64
NCORES = 8
NH = 4          # heads per core
NKB = S // 128  # 16 key blocks
NQC = S // 512  # 4 query chunks
KC = DM // 128  # 8 contraction chunks

# RoPE partner permutation: place original dim d so that partner(p) = p ^ 16
# (within a 32-partition quadrant, reachable by DVE stream_shuffle).
ROPE_PERM = []
for _p in range(64):
    q32, r32 = _p // 32, _p % 32
    ROPE_PERM.append(q32 * 16 + r32 if r32 < 16 else 32 + q32 * 16 + (r32 - 16))
ROPE_SGN = np.array([-1.0 if (p % 32) < 16 else 1.0 for p in range(64)], np.float32)
SHUF_MASK = [i ^ 16 for i in range(32)]

_CACHED = {}


def build_program():
    nc = bacc.Bacc(None, target_bir_lowering=False)
    qT_d = nc.declare_dram_parameter("qT", [DM, S], BF16, isOutput=False)
    wqk_d = nc.declare_dram_parameter("wqkT", [DM, 512], BF16, isOutput=False)
    wv_d = nc.declare_dram_parameter("wvT", [DM, 256], BF16, isOutput=False)
    cos_d = nc.declare_dram_parameter("cosT", [128, S], BF16, isOutput=False)
    sin_d = nc.declare_dram_parameter("sinT", [128, S], BF16, isOutput=False)
    mkv_d = nc.declare_dram_parameter("maskv", [128, NKB], F32, isOutput=False)
    wo_d = nc.declare_dram_parameter("woutT", [256, DM], BF16, isOutput=False)
    yT_d = nc.declare_dram_parameter("yT", [DM, S], F32, isOutput=True)
    dscr = nc.dram_tensor("den_scratch", [16, 512], F32)
    dscr2 = nc.dram_tensor("rcp_scratch", [16, 512], F32)

    with tile.TileContext(nc) as tc:
        with (
            tc.tile_pool(name="const", bufs=1) as cpool,
            tc.tile_pool(name="work", bufs=1) as wpool,
            tc.tile_pool(name="rope", bufs=3) as rpool,
            tc.tile_pool(name="pt", bufs=6) as ptpool,
            tc.tile_pool(name="nrm", bufs=4) as npool,
            tc.tile_pool(name="yout", bufs=2) as ypool,
            tc.tile_pool(name="psA", bufs=2, space="PSUM") as psA,
            tc.tile_pool(name="psP", bufs=2, space="PSUM") as psP,
            tc.tile_pool(name="psO", bufs=2, space="PSUM") as psO,
        ):
            qT_sb = cpool.tile([128, KC, S], BF16, tag="qT")
            wqk_sb = cpool.tile([128, KC, 512], BF16, tag="wqk")
            wv_sb = cpool.tile([128, KC, 256], BF16, tag="wv")
            cos_sb = cpool.tile([128, S], BF16, tag="cos")
            sin_sb = cpool.tile([128, S], BF16, tag="sin")
            mkv_sb = cpool.tile([128, NKB], F32, tag="mkv")
            wo_sb = cpool.tile([128, 2, DM], BF16, tag="wo")

            qk_sb = wpool.tile([128, 4, S], BF16, tag="qk")
            vaug_sb = wpool.tile([128, NKB, 4, 128], BF16, tag="vaug")
            ohT_sb = wpool.tile([128, 2, S], BF16, tag="ohT")

            nc.sync.dma_start(mkv_sb[:], mkv_d[:])
            nc.sync.dma_start(wqk_sb[:], wqk_d.rearrange("(c p) s -> p c s", p=128))
            qT_r = qT_d.rearrange("(c p) s -> p c s", p=128)
            for kc in range(KC):
                nc.sync.dma_start(qT_sb[:, kc, :], qT_r[:, kc, :])
            nc.sync.dma_start(wv_sb[:], wv_d.rearrange("(c p) s -> p c s", p=128))
            nc.sync.dma_start(cos_sb[:], cos_d[:])
            nc.sync.dma_start(sin_sb[:], sin_d[:])
            nc.sync.dma_start(wo_sb[:], wo_d.rearrange("(c p) s -> p c s", p=128))

            nc.gpsimd.memset(vaug_sb[:], 0.0)
            # mask columns of v_aug: even slots col 64, odd slots col 32
            # (den must land on a legal engine start partition: 0/32/64/96)
            mkv_col = mkv_sb.rearrange("p (k o) -> p k o", o=1)
            nc.gpsimd.tensor_copy(vaug_sb[:, :, 0, 64:65], mkv_col)
            nc.gpsimd.tensor_copy(vaug_sb[:, :, 2, 64:65], mkv_col)
            nc.gpsimd.tensor_copy(vaug_sb[:, :, 1, 32:33], mkv_col)
            nc.gpsimd.tensor_copy(vaug_sb[:, :, 3, 32:33], mkv_col)

            def emit_qk(mt, qn):
                """project + rope one [128, 512] chunk of q or k (pair of heads)"""
                qsl = slice(qn * 512, qn * 512 + 512)
                ps = psP.tile([128, 512], F32, tag="psP")
                for kc in range(KC):
                    nc.tensor.matmul(
                        ps[:],
                        lhsT=wqk_sb[:, kc, mt * 128:(mt + 1) * 128],
                        rhs=qT_sb[:, kc, qsl],
                        start=(kc == 0),
                        stop=(kc == KC - 1),
                    )
                qkp = rpool.tile([128, 512], BF16, tag="qkp")
                nc.vector.tensor_copy(qkp[:], ps[:])
                shuf = rpool.tile([128, 512], BF16, tag="shuf")
                nc.vector.stream_shuffle(shuf[:], qkp[:], mask=SHUF_MASK)
                t1 = rpool.tile([128, 512], BF16, tag="t1")
                nc.vector.tensor_mul(t1[:], qkp[:], cos_sb[:, qsl])
                t2 = rpool.tile([128, 512], BF16, tag="t2")
                nc.vector.tensor_mul(t2[:], shuf[:], sin_sb[:, qsl])
                nc.vector.tensor_add(qk_sb[:, mt, qsl], t1[:], t2[:])

            def emit_v(st):
                """project + mask one [128 keys, 4*64] v block into v_aug"""
                ps = psP.tile([128, 512], F32, tag="psP")
                psv = ps[:, 0:256]
                for kc in range(KC):
                    nc.tensor.matmul(
                        psv,
                        lhsT=qT_sb[:, kc, st * 128:(st + 1) * 128],
                        rhs=wv_sb[:, kc, :],
                        start=(kc == 0),
                        stop=(kc == KC - 1),
                    )
                psv_h = psv.rearrange("p (h d) -> p h d", h=4)
                msk = mkv_sb[:, st:st + 1]
                # even local heads (slots 0,2) -> cols 0:64 ; odd -> cols 64:128
                nc.vector.tensor_scalar_mul(vaug_sb[:, st, 0:4:2, 0:64], psv_h[:, 0:4:2, :], msk)
                nc.vector.tensor_scalar_mul(vaug_sb[:, st, 1:4:2, 64:128], psv_h[:, 1:4:2, :], msk)

            def emit_attn(pair, qc):
                nkb = 4 * qc + 4
                qmt, kmt = pair, 2 + pair
                qsl = slice(qc * 512, qc * 512 + 512)
                oT = [psO.tile([128, 512], F32, tag="psO", name=f"oT{_h}") for _h in range(2)]
                for kb in range(nkb):
                    ksl = slice(kb * 128, kb * 128 + 128)
                    st_ps = psA.tile([128, 1024], F32, tag="psA", name="stps")
                    for h in range(2):
                        pr = slice(64 * h, 64 * h + 64)
                        nc.tensor.matmul(
                            st_ps[:, h * 512:(h + 1) * 512],
                            lhsT=qk_sb[pr, kmt, ksl],
                            rhs=qk_sb[pr, qmt, qsl],
                            start=True,
                            stop=True,
                            skip_group_check=True,
                        )
                    pt = ptpool.tile([128, 1024], BF16, tag="pt", name="pt")
                    nc.scalar.activation(
                        pt[:], st_ps[:],
                        mybir.ActivationFunctionType.Exp,
                        scale=0.125,
                    )
                    joff = kb - 4 * qc
                    if joff >= 0:
                        co = joff * 128
                        for h in range(2):
                            if co > 0:
                                nc.gpsimd.memset(pt[:, h * 512:h * 512 + co], 0.0)
                            nc.gpsimd.affine_select(
                                pt[:, h * 512 + co:h * 512 + co + 128],
                                pt[:, h * 512 + co:h * 512 + co + 128],
                                pattern=[[1, 128]],
                                compare_op=mybir.AluOpType.is_ge,
                                fill=0.0,
                                base=0,
                                channel_multiplier=-1,
                            )
                    for h in range(2):
                        nc.tensor.matmul(
                            oT[h][:],
                            lhsT=vaug_sb[:, kb, 2 * pair + h, :],
                            rhs=pt[:, h * 512:(h + 1) * 512],
                            start=(kb == 0),
                            stop=(kb == nkb - 1),
                            skip_group_check=True,
                        )
                base = (pair * 4 + qc) * 2
                osb = []
                for h in range(2):
                    den_row = 64 if h == 0 else 32
                    o = npool.tile([128, 512], F32, tag="osb", name=f"osb{h}")
                    nc.vector.tensor_copy(o[:], oT[h][:])  # frees the psum bank
                    osb.append(o)
                    nc.sync.dma_start(dscr[base + h:base + h + 1, :], o[den_row:den_row + 1, :])
                # reciprocal on a partition-packed view (DMA reshape through
                # DRAM): [2, 512] dens -> [128, 8] -> 1/x -> back, then
                # partition-broadcast each head's 512 recips to 64 rows.
                rcp = npool.tile([128, 8], F32, tag="rcp")
                nc.sync.dma_start(rcp[:], dscr[base:base + 2, :].rearrange("a (p f) -> (a p) f", f=8))
                rcp2 = npool.tile([128, 8], F32, tag="rcp2")
                nc.vector.reciprocal(rcp2[:], rcp[:])
                nc.sync.dma_start(dscr2[base:base + 2, :].rearrange("a (p f) -> (a p) f", f=8), rcp2[:])
                bc = npool.tile([128, 512], F32, tag="bc")
                nc.sync.dma_start(bc[0:64, :], dscr2[base:base + 1, :].to_broadcast((64, 512)))
                nc.sync.dma_start(bc[64:128, :], dscr2[base + 1:base + 2, :].to_broadcast((64, 512)))
                nc.vector.tensor_mul(ohT_sb[0:64, pair, qsl], osb[0][0:64, :], bc[0:64, :])
                nc.vector.tensor_mul(ohT_sb[64:128, pair, qsl], osb[1][64:128, :], bc[64:128, :])

            def emit_outproj(qn):
                qsl = slice(qn * 512, qn * 512 + 512)
                y = ypool.tile([128, 8, 512], F32, tag="y")
                for mt in range(8):
                    ps = psP.tile([128, 512], F32, tag="psP")
                    for kc2 in range(2):
                        nc.tensor.matmul(
                            ps[:],
                            lhsT=wo_sb[:, kc2, mt * 128:(mt + 1) * 128],
                            rhs=ohT_sb[:, kc2, qsl],
                            start=(kc2 == 0),
                            stop=(kc2 == 1),
                        )
                    nc.vector.tensor_copy(y[:, mt, :], ps[:])
                    if mt == 3:
                        nc.sync.dma_start(yT_r[:, 0:4, qsl], y[:, 0:4, :])
                nc.sync.dma_start(yT_r[:, 4:8, qsl], y[:, 4:8, :])

            # pipeline by query chunk.  Emission order = scheduling
            # priority: next-chunk projections go between the two attention
            # blocks so they execute during the exp-paced stream (PE filler)
            # and are ready the moment the previous chunk drains; the
            # out-projection fills the second half of each round.
            yT_r = yT_d.rearrange("(c p) s -> p c s", p=128)

            def emit_prep(qc):
                emit_qk(2, qc)
                emit_qk(3, qc)
                emit_qk(0, qc)
                emit_qk(1, qc)
                for st in range(4 * qc, 4 * qc + 4):
                    emit_v(st)

            emit_prep(0)
            for qc in range(NQC):
                emit_attn(0, qc)
                if qc + 1 < NQC:
                    emit_prep(qc + 1)
                emit_attn(1, qc)
                emit_outproj(qc)

    nc.compile()
    return nc


def make_in_maps(query, W_in, W_out, sin_q, cos_q, attn_mask):
    bf = ml_dtypes.bfloat16
    cosT = np.asarray(cos_q, np.float32)[0, 0].T  # [64, S]
    sinT = np.asarray(sin_q, np.float32)[0, 0].T
    cosT_p = cosT[ROPE_PERM]
    sinT_p = sinT[ROPE_PERM] * ROPE_SGN[:, None]
    cos2 = np.concatenate([cosT_p, cosT_p], 0).astype(bf)    # [128, S]
    sin2 = np.concatenate([sinT_p, sinT_p], 0).astype(bf)
    W_in = np.asarray(W_in, np.float32)
    W_out = np.asarray(W_out, np.float32)
    query = np.asarray(query, np.float32)
    attn_mask = np.asarray(attn_mask)

    in_maps = []
    for c in range(NCORES):
        b, g = c // 4, c % 4
        heads = range(4 * g, 4 * g + 4)
        qrows = np.concatenate([W_in[h * 64:(h + 1) * 64][ROPE_PERM] for h in heads])
        krows = np.concatenate([W_in[TD + h * 64:TD + (h + 1) * 64][ROPE_PERM] for h in heads])
        vrows = np.concatenate([W_in[2 * TD + h * 64:2 * TD + (h + 1) * 64] for h in heads])
        tcols = np.concatenate([np.arange(h * 64, (h + 1) * 64) for h in heads])
        in_maps.append({
            "qT": np.ascontiguousarray(query[b].T).astype(bf),
            "wqkT": np.ascontiguousarray(np.concatenate([qrows, krows], 0).T).astype(bf),
            "wvT": np.ascontiguousarray(vrows.T).astype(bf),
            "cosT": cos2,
            "sinT": sin2,
            "maskv": np.ascontiguousarray(
                attn_mask[b].astype(np.float32).reshape(NKB, 128).T),
            "woutT": np.ascontiguousarray(W_out[:, tcols].T).astype(bf),
        })
    return in_maps


def _ensure_ntff_hook():
    """The image's antenv lacks axon_hooks; supply it so trace=True works."""
    try:
        from antenv.axon_hooks import get_axon_ntff_profile_hook  # noqa: F401
        return
    except ImportError:
        pass
    import types

    if "/root/.axon_site" not in sys.path:
        sys.path.insert(0, "/root/.axon_site")
    from trn_agent_boot.trn_boot import _ntff_profile_via_ctypes

    hook = _ntff_profile_via_ctypes("/opt/axon/libaxon_pjrt.so")
    mod = types.ModuleType("antenv.axon_hooks")
    mod._hook = hook
    mod.get_axon_ntff_profile_hook = lambda: mod._hook
    mod.set_axon_ntff_profile_hook = lambda h: setattr(mod, "_hook", h)
    sys.modules["antenv.axon_hooks"] = mod
    import antenv

    antenv.axon_hooks = mod


def kernel(query, W_in, W_out, sin_q, cos_q, attn_mask):
    if "nc" not in _CACHED:
        _CACHED["nc"] = build_program()
    nc = _CACHED["nc"]
    in_maps = make_in_maps(query, W_in, W_out, sin_q, cos_q, attn_mask)

    from concourse.bass_utils import run_bass_kernel_spmd

    trace = bool(os.environ.get("KERNEL_PROFILE"))
    if trace:
        try:
            _ensure_ntff_hook()
        except Exception as e:  # profiling is best-effort
            print(f"ntff hook unavailable: {e}")
            trace = False
    try:
        res = run_bass_kernel_spmd(nc, in_maps, list(range(NCORES)), trace=trace)
    except Exception:
        if not trace:
            raise
        res = run_bass_kernel_spmd(nc, in_maps, list(range(NCORES)), trace=False)
    _CACHED["last_result"] = res

    y = np.zeros((B, S, DM), np.float32)
    for c in range(NCORES):
        y[c // 4] += res.results[c]["yT"].T
    return y
